# revision 14
# baseline (speedup 1.0000x reference)
"""Trainium2 Bass kernel for nn_DemonstrationAttentionQModel.

Key algebraic facts exploited (all exact):
  - Only demonstration[0] and demonstration_length[0] are used.
  - scores = enc@Wd + b + (h@Wh) : the h term is a per-batch scalar added
    uniformly across L, and softmax is shift-invariant => attention weights
    are constant over decoder time.
  - Hence decoder inputs x_t = relu(comb([attn, obs_t])) are precomputable;
    only the LSTM h/c recurrences (80 + 40 steps) are sequential.
  - mid/out are linear-linear => fused into one [18,1024] matmul at the end.

Distribution: Megatron-style gate split over 8 cores. Core c owns h-dims
[128c,128c+128) (gate rows reordered [i,f,o,g]); per step it computes its 512
gates (weights as the f32r moving operand, N=512), does the cell elementwise,
PE-transposes its h-chunk and AllGathers the 8 chunks into the full hT.

kernel(**inputs) -> (q [T,B,A], h [B,H], c [B,H]) float32.
"""

import os
import sys

sys.path.insert(0, "/opt/trn_rl_repo")

import numpy as np

import concourse.bass as bass  # noqa
import concourse.bacc as bacc
import concourse.mybir as mybir
import concourse.tile as tile

dt = mybir.dt
Act = mybir.ActivationFunctionType
Alu = mybir.AluOpType
Ax = mybir.AxisListType

NC = 8
T, B, L = 40, 64, 80
DIN = 256
H = 1024
A = 18
HC = H // NC          # 128
GC = 4 * HC           # 512
KH = H // 128         # 8
KD = DIN // 128       # 2
NEG = -1e30

F32 = dt.float32
F32R = dt.float32r

_CACHE = {}


# ---------------------------------------------------------------- host prep

def _gate_rows(c):
    """Gate rows of core c, reordered [i, f, o, g] (PyTorch order is i,f,g,o)."""
    hs = np.arange(HC) + HC * c
    return np.concatenate([hs, H + hs, 3 * H + hs, 2 * H + hs])


def _pack_rhs(w_T):
    """[K, N] -> [128, (K//128)*N], k-tile t at cols [N*t : N*(t+1)]."""
    K, N = w_T.shape
    k = K // 128
    return np.ascontiguousarray(
        w_T.reshape(k, 128, N).transpose(1, 0, 2).reshape(128, k * N)
    )


def _prep_inputs(inputs):
    f = lambda x: np.asarray(x, dtype=np.float32)
    state = f(inputs["state"])                     # [T,B,DIN]
    demo0 = f(inputs["demonstration"][0])          # [B,L,DIN]
    lengths = np.asarray(inputs["demonstration_length"][0], dtype=np.int64)
    h0, c0 = f(inputs["h0"]), f(inputs["c0"])

    enc_Wih, enc_Whh = f(inputs["enc_Wih"]), f(inputs["enc_Whh"])
    enc_b = f(inputs["enc_bih"]) + f(inputs["enc_bhh"])
    attn_W, attn_b = f(inputs["attn_W"]), f(inputs["attn_b"])
    comb_W, comb_b = f(inputs["comb_W"]), f(inputs["comb_b"])
    lstm_Wih, lstm_Whh = f(inputs["lstm_Wih"]), f(inputs["lstm_Whh"])
    lstm_b = f(inputs["lstm_bih"]) + f(inputs["lstm_bhh"])
    mid_W, mid_b = f(inputs["mid_W"]), f(inputs["mid_b"])
    out_W, out_b = f(inputs["out_W"]), f(inputs["out_b"])

    W_qo = out_W @ mid_W                   # [A, H]
    b_qo = out_W @ mid_b + out_b           # [A]

    xe = demo0.transpose(1, 0, 2).reshape(L * B, DIN)      # (l*B+b, DIN)
    xe_T = _pack_rhs(np.ascontiguousarray(xe.T))
    st = state.reshape(T * B, DIN)                         # (t*B+b, DIN)
    st_T = _pack_rhs(np.ascontiguousarray(st.T))
    penalty = np.where(
        np.arange(L)[None, :] < lengths[:, None], attn_b[0], NEG
    ).astype(np.float32)                                    # [B,L]
    h0T = np.ascontiguousarray(h0.T)                        # [H,B]
    Wd = attn_W[0, :H]

    in_maps = []
    for c in range(NC):
        rows = _gate_rows(c)
        hs = slice(HC * c, HC * (c + 1))
        in_maps.append({
            "enc_whhT": _pack_rhs(np.ascontiguousarray(enc_Whh[rows].T)),
            "lstm_whhT": _pack_rhs(np.ascontiguousarray(lstm_Whh[rows].T)),
            "enc_wihT": _pack_rhs(np.ascontiguousarray(enc_Wih[rows].T)),
            "lstm_wihT": _pack_rhs(np.ascontiguousarray(lstm_Wih[rows].T)),
            "combT": _pack_rhs(np.ascontiguousarray(comb_W[hs].T)),
            "wqoT": np.ascontiguousarray(W_qo[:, hs].T),
            "wd": np.ascontiguousarray(Wd[hs][:, None]),
            "enc_bias": np.broadcast_to(enc_b[rows], (128, GC)).copy(),
            "lstm_bias": np.broadcast_to(lstm_b[rows], (128, GC)).copy(),
            "comb_bias": np.ascontiguousarray(comb_b[hs][:, None]),
            "penalty": penalty,
            "xe_T": xe_T,
            "st_T": st_T,
            "h0T": _pack_rhs(h0T),
            "c0c": np.ascontiguousarray(c0[:, hs]),
            "ident": np.eye(128, dtype=np.float32),
        })
    return in_maps, b_qo


# ------------------------------------------------------------- device program

_INPUT_SPECS = [
    ("enc_whhT", [128, KH * GC], F32R),
    ("lstm_whhT", [128, KH * GC], F32R),
    ("enc_wihT", [128, KD * GC], F32R),
    ("lstm_wihT", [128, KH * GC], F32R),
    ("combT", [128, 10 * HC], F32R),
    ("wqoT", [HC, A], F32R),
    ("wd", [HC, 1], F32R),
    ("enc_bias", [128, GC], F32),
    ("lstm_bias", [128, GC], F32),
    ("comb_bias", [HC, 1], F32),
    ("penalty", [B, L], F32),
    ("xe_T", [128, KD * L * B], F32R),
    ("st_T", [128, KD * T * B], F32R),
    ("h0T", [128, KH * B], F32R),
    ("c0c", [B, HC], F32),
    ("ident", [128, 128], F32),
]

_OUTPUT_SPECS = [
    ("q_part", [A, T * B], F32),
    ("h_out", [HC, B], F32),
    ("c_out", [B, HC], F32),
]

RG = [list(range(NC))]


def _build_program():
    nc = bacc.Bacc("TRN2", target_bir_lowering=False, debug=False,
                   num_devices=NC)
    io = {}
    for name, shape, d in _INPUT_SPECS:
        io[name] = nc.dram_tensor(name, shape, d, kind="ExternalInput").ap()
    for name, shape, d in _OUTPUT_SPECS:
        io[name] = nc.dram_tensor(name, shape, d, kind="ExternalOutput").ap()

    # internal DRAM
    prei_d = nc.dram_tensor("prei_d", [L * B // 128, 128, GC], F32)
    prex_d = nc.dram_tensor("prex_d", [T * B // 128, 128, GC], F32)
    scores_d = nc.dram_tensor("scores_d", [L, B], F32)
    scores_r = nc.dram_tensor("scores_r", [L, B], F32, addr_space="Shared")
    arch_d = nc.dram_tensor("arch_d", [B, L, HC], F32)

    import contextlib
    with tile.TileContext(nc) as tc, contextlib.ExitStack() as ctx:
        persist = ctx.enter_context(tc.tile_pool(name="persist", bufs=1))
        psum = ctx.enter_context(tc.tile_pool(name="psum", bufs=4, space="PSUM"))
        cell = ctx.enter_context(tc.tile_pool(name="cell", bufs=8))
        prep = ctx.enter_context(tc.tile_pool(name="prep", bufs=3))
        dram = ctx.enter_context(tc.tile_pool(name="dram", bufs=3, space="DRAM"))

        def ptile(shape, name):
            return psum.tile(shape, F32, tag="ps", name=name)

        def ctile(shape, name, dtype=F32):
            return cell.tile(shape, dtype, tag="cl", name=name)

        # --- persistent smalls: two merged tiles + per-core state ---
        # smalls_r (f32r): wd [HC,1] at col 0; wqoT [HC,A] at cols 1:1+A
        smalls_r = persist.tile([128, 1 + A], F32R, tag="smalls_r",
                                name="smalls_r")
        nc.sync.dma_start(smalls_r[:HC, 0:1], io["wd"])
        nc.sync.dma_start(smalls_r[:HC, 1:1 + A], io["wqoT"])
        wd = smalls_r[:HC, 0:1]
        wqoT = smalls_r[:HC, 1:1 + A]
        # smalls_f (f32): comb_bias 1 | ident 128 | penalty 80 | enc_bias 512
        #                 | lstm_bias 512
        smalls_f = persist.tile([128, 1 + 128 + L + GC + GC], F32,
                                tag="smalls_f", name="smalls_f")
        nc.sync.dma_start(smalls_f[:HC, 0:1], io["comb_bias"])
        nc.sync.dma_start(smalls_f[:, 1:129], io["ident"])
        nc.sync.dma_start(smalls_f[:B, 129:129 + L], io["penalty"])
        nc.sync.dma_start(smalls_f[:, 209:209 + GC], io["enc_bias"])
        nc.sync.dma_start(smalls_f[:, 209 + GC:209 + 2 * GC], io["lstm_bias"])
        comb_bias = smalls_f[:HC, 0:1]
        ident = smalls_f[:, 1:129]
        penalty = smalls_f[:B, 129:129 + L]
        enc_bias = smalls_f[:, 209:209 + GC]
        lstm_bias = smalls_f[:, 209 + GC:209 + 2 * GC]

        hT_all = persist.tile([128, KH * B], F32R, tag="hT_all", name="hT_all")
        c_chunk = persist.tile([B, HC], F32, tag="c_chunk", name="c_chunk")
        attnT = persist.tile([128, KH * B], F32R, tag="attnT", name="attnT")
        arch_decT = persist.tile([HC, T * B], F32R, tag="arch_decT",
                                 name="arch_decT")
        h_bp_fin = persist.tile([B, HC], F32, tag="h_bp_fin", name="h_bp_fin")

        def lstm_step(step, whhT, pre_d, h_bp_dest, arch_T_dest, do_ag,
                      score_dst=None, arch_dram=None):
            g, off = divmod(step, 2)
            pre = prep.tile([B, GC], F32, tag="pre", name="pre")
            nc.sync.dma_start(pre[:], pre_d[g, 64 * off:64 * off + B, :])
            psum_g = ptile([B, GC], "psum_g")
            for k in range(KH):
                nc.tensor.matmul(
                    psum_g[:], hT_all[:, B * k:B * (k + 1)],
                    whhT[:, GC * k:GC * (k + 1)],
                    start=(k == 0), stop=(k == KH - 1),
                )
            # add input-side gates in place (PSUM); split so the sigmoid
            # starts while the tanh-side add is still running on DVE
            nc.vector.scalar_tensor_tensor(
                psum_g[:, :3 * HC], psum_g[:, :3 * HC], 0.0, pre[:, :3 * HC],
                Alu.add, Alu.add)
            nc.vector.scalar_tensor_tensor(
                psum_g[:, 3 * HC:], psum_g[:, 3 * HC:], 0.0, pre[:, 3 * HC:],
                Alu.add, Alu.add)
            sifo = ctile([B, 3 * HC], "sifo")
            nc.scalar.activation(sifo[:], psum_g[:, :3 * HC], Act.Sigmoid)
            tg = ctile([B, HC], "tg")
            nc.scalar.activation(tg[:], psum_g[:, 3 * HC:], Act.Tanh)
            t1 = ctile([B, HC], "t1")
            nc.vector.scalar_tensor_tensor(
                t1[:], sifo[:, HC:2 * HC], 0.0, c_chunk[:], Alu.add, Alu.mult)
            t2 = ctile([B, HC], "t2")
            nc.vector.scalar_tensor_tensor(
                t2[:], sifo[:, :HC], 0.0, tg[:], Alu.add, Alu.mult)
            nc.vector.scalar_tensor_tensor(
                c_chunk[:], t1[:], 0.0, t2[:], Alu.add, Alu.add)
            tc_ = ctile([B, HC], "tc_")
            nc.scalar.activation(tc_[:], c_chunk[:], Act.Tanh)
            nc.vector.scalar_tensor_tensor(
                h_bp_dest, sifo[:, 2 * HC:], 0.0, tc_[:], Alu.add, Alu.mult)
            if arch_dram is not None:
                nc.sync.dma_start(arch_dram, h_bp_dest)
            psum_hT = ptile([HC, B], "psum_hT")
            nc.tensor.transpose(psum_hT[:], h_bp_dest, ident[:B, :B])
            nc.vector.tensor_copy(arch_T_dest, psum_hT[:])
            if score_dst is not None:
                psum_sc = ptile([1, B], "psum_sc")
                nc.tensor.matmul(psum_sc[:], wd, arch_T_dest,
                                 start=True, stop=True)
                sc_sb = ctile([1, B], "sc_sb")
                nc.vector.tensor_copy(sc_sb[:], psum_sc[:])
                nc.sync.dma_start(score_dst, sc_sb[:])
            if do_ag:
                cc_in = dram.tile([HC, B], F32R, tag="cc_in", name="cc_in")
                cc_out = dram.tile([H, B], F32R, tag="cc_out", name="cc_out",
                                   addr_space="Shared")
                nc.sync.dma_start(cc_in[:], arch_T_dest)
                nc.gpsimd.collective_compute(
                    "AllGather", Alu.bypass, ins=[cc_in.opt()],
                    outs=[cc_out.opt()], replica_groups=RG)
                nc.sync.dma_start(
                    hT_all[:].rearrange("p (k n) -> p k n", n=B),
                    cc_out[:].rearrange("(k p) n -> p k n", p=128))

        # ================= encoder =================
        with tc.tile_pool(name="enc_phase", bufs=1) as encp:
            enc_whhT = encp.tile([128, KH * GC], F32R, tag="enc_whhT",
                                 name="enc_whhT")
            nc.sync.dma_start(enc_whhT[:], io["enc_whhT"])

            # --- PreI -> DRAM ---
            with tc.tile_pool(name="xe_pool", bufs=1) as xep:
                enc_wihT = xep.tile([128, KD * GC], F32R, tag="enc_wihT",
                                    name="enc_wihT")
                nc.sync.dma_start(enc_wihT[:], io["enc_wihT"])
                xe_sb = xep.tile([128, KD * L * B], F32R, tag="xe_sb",
                                 name="xe_sb")
                nc.sync.dma_start(xe_sb[:], io["xe_T"])
                for g in range(L * B // 128):            # 40 groups
                    ps = ptile([128, GC], "ps_pre")
                    for k in range(KD):
                        nc.tensor.matmul(
                            ps[:],
                            xe_sb[:, L * B * k + 128 * g:L * B * k + 128 * (g + 1)],
                            enc_wihT[:, GC * k:GC * (k + 1)],
                            start=(k == 0), stop=(k == KD - 1),
                        )
                    stg = prep.tile([128, GC], F32, tag="stg", name="stg")
                    nc.vector.scalar_tensor_tensor(
                        stg[:], ps[:], 0.0, enc_bias, Alu.add, Alu.add)
                    nc.sync.dma_start(prei_d[g], stg[:])

            # --- encoder recurrence ---
            nc.vector.memset(hT_all[:].bitcast(F32), 0.0)
            nc.vector.memset(c_chunk[:], 0.0)
            for l in range(L):
                stageT = ctile([HC, B], "stageT", F32R)
                h_bp = ctile([B, HC], "h_bp")
                lstm_step(
                    l, enc_whhT, prei_d,
                    h_bp_dest=h_bp[:],
                    arch_T_dest=stageT[:],
                    do_ag=(l < L - 1),
                    score_dst=scores_d[l],
                    arch_dram=arch_d[:, l, :],
                )

            # --- scores AllReduce + softmax ---
            with tc.tile_pool(name="smx", bufs=4) as smx:
                def stile(shape, name):
                    return smx.tile(shape, F32, tag="sm", name=name)

                nc.gpsimd.collective_compute(
                    "AllReduce", Alu.add, ins=[scores_d.ap().opt()],
                    outs=[scores_r.ap().opt()], replica_groups=RG)
                sc_lb = stile([L, B], "sc_lb")
                nc.sync.dma_start(sc_lb[:], scores_r.ap())
                ps_sc = ptile([B, L], "ps_scT")
                nc.tensor.transpose(ps_sc[:], sc_lb[:], ident[:L, :L])
                scores_bp = stile([B, L], "scores_bp")
                nc.vector.scalar_tensor_tensor(
                    scores_bp[:], ps_sc[:], 0.0, penalty, Alu.add, Alu.add)
                mx = stile([B, 1], "mx")
                nc.vector.tensor_reduce(mx[:], scores_bp[:], Ax.X, Alu.max)
                nmx = stile([B, 1], "nmx")
                nc.vector.tensor_scalar_mul(nmx[:], mx[:], -1.0)
                wexp = stile([B, L], "wexp")
                sumexp = stile([B, 1], "sumexp")
                nc.scalar.activation(wexp[:], scores_bp[:], Act.Exp,
                                     bias=nmx[:], scale=1.0,
                                     accum_out=sumexp[:])
                rsum = stile([B, 1], "rsum")
                nc.vector.reciprocal(rsum[:], sumexp[:])
                wat = stile([B, L], "wat")
                nc.vector.tensor_scalar(wat[:], wexp[:], rsum[:], None,
                                        Alu.mult)

                # --- attention in L-chunks from the DRAM archive ---
                LC = 20
                attn_acc = stile([B, HC], "attn_acc")
                for q in range(L // LC):
                    aq = smx.tile([B, LC * HC], F32, tag="aq", name="aq",
                                  bufs=2)
                    nc.sync.dma_start(
                        aq[:].rearrange("b (l h) -> b l h", l=LC),
                        arch_d[:, LC * q:LC * (q + 1), :])
                    pq = smx.tile([B, LC * HC], F32, tag="pq", name="pq",
                                  bufs=2)
                    nc.vector.scalar_tensor_tensor(
                        pq[:].rearrange("b (h l) -> b l h", l=LC),
                        aq[:].rearrange("b (l h) -> b l h", l=LC),
                        0.0,
                        wat[:, LC * q:LC * (q + 1)].rearrange(
                            "b (l o) -> b l o", o=1).broadcast_to([B, LC, HC]),
                        Alu.add, Alu.mult)
                    part = stile([B, HC], "attn_part")
                    nc.vector.tensor_reduce(
                        part[:], pq[:].rearrange("b (h l) -> b h l", l=LC),
                        Ax.X, Alu.add)
                    if q == 0:
                        nc.vector.tensor_copy(attn_acc[:], part[:])
                    else:
                        nc.vector.scalar_tensor_tensor(
                            attn_acc[:], attn_acc[:], 0.0, part[:],
                            Alu.add, Alu.add)
                ps_at = ptile([HC, B], "ps_at")
                nc.tensor.transpose(ps_at[:], attn_acc[:], ident[:B, :B])
                attn_stage = smx.tile([HC, B], F32R, tag="sm",
                                      name="attn_stage")
                nc.vector.tensor_copy(attn_stage[:], ps_at[:])
                at_in = dram.tile([HC, B], F32R, tag="at_in", name="at_in")
                at_out = dram.tile([H, B], F32R, tag="at_out", name="at_out",
                                   addr_space="Shared")
                nc.sync.dma_start(at_in[:], attn_stage[:])
                nc.gpsimd.collective_compute(
                    "AllGather", Alu.bypass, ins=[at_in.opt()],
                    outs=[at_out.opt()], replica_groups=RG)
                nc.sync.dma_start(
                    attnT[:].rearrange("p (k n) -> p k n", n=B),
                    at_out[:].rearrange("(k p) n -> p k n", p=128))

        # ================= decoder-side weights =================
        lstm_whhT = persist.tile([128, KH * GC], F32R, tag="lstm_whhT",
                                 name="lstm_whhT")
        nc.sync.dma_start(lstm_whhT[:], io["lstm_whhT"])

        # ================= x / PreX precompute =================
        x_out = dram.tile([H, T * B], F32R, tag="x_out", name="x_out",
                      addr_space="Shared")
        with tc.tile_pool(name="x_pool", bufs=1) as xp:
            combT = xp.tile([128, 10 * HC], F32R, tag="combT", name="combT")
            nc.sync.dma_start(combT[:], io["combT"])
            st_sb = xp.tile([128, KD * T * B], F32R, tag="st_sb", name="st_sb")
            nc.sync.dma_start(st_sb[:], io["st_T"])
            ps_pa = ptile([HC, B], "ps_pa")
            for k in range(KH):
                nc.tensor.matmul(
                    ps_pa[:], combT[:, HC * k:HC * (k + 1)],
                    attnT[:, B * k:B * (k + 1)],
                    start=(k == 0), stop=(k == KH - 1))
            part_a = xp.tile([HC, B], F32, tag="part_a", name="part_a")
            nc.vector.tensor_copy(part_a[:], ps_pa[:])
            xT_sb = xp.tile([HC, T * B], F32R, tag="xT_sb", name="xT_sb")
            for n in range(T * B // 512):                # 5 chunks
                ps_x = ptile([HC, 512], "ps_x")
                for k in range(KD):
                    nc.tensor.matmul(
                        ps_x[:],
                        combT[:, HC * (KH + k):HC * (KH + k + 1)],
                        st_sb[:, T * B * k + 512 * n:T * B * k + 512 * (n + 1)],
                        start=(k == 0), stop=(k == KD - 1))
                xsum = xp.tile([HC, 512], F32, tag="xsum", name="xsum", bufs=2)
                nc.vector.scalar_tensor_tensor(
                    xsum[:].rearrange("p (t b) -> p t b", b=B),
                    ps_x[:].rearrange("p (t b) -> p t b", b=B),
                    0.0,
                    part_a[:].rearrange("p (o b) -> p o b", o=1).broadcast_to(
                        [HC, 512 // B, B]),
                    Alu.add, Alu.add)
                nc.scalar.activation(
                    xT_sb[:, 512 * n:512 * (n + 1)], xsum[:], Act.Relu,
                    bias=comb_bias, scale=1.0)
            x_in = dram.tile([HC, T * B], F32R, tag="x_in", name="x_in")
            nc.sync.dma_start(x_in[:], xT_sb[:])
            nc.gpsimd.collective_compute(
                "AllGather", Alu.bypass, ins=[x_in.opt()], outs=[x_out.opt()],
                replica_groups=RG)

        with tc.tile_pool(name="px_pool", bufs=1) as pxp:
            lstm_wihT = pxp.tile([128, KH * GC], F32R, tag="lstm_wihT",
                                 name="lstm_wihT")
            nc.sync.dma_start(lstm_wihT[:], io["lstm_wihT"])
            # PreX in 5 supergroups of 4 M-groups (512 (t,b) rows each)
            for sg in range(T * B // 512):
                xg = pxp.tile([128, KH * 512], F32R, tag="xg", name="xg",
                              bufs=2)
                nc.sync.dma_start(
                    xg[:].rearrange("p (k n) -> p k n", n=512),
                    x_out[:, 512 * sg:512 * (sg + 1)].rearrange(
                        "(k p) n -> p k n", p=128))
                for g4 in range(4):
                    g = 4 * sg + g4
                    ps = ptile([128, GC], "ps_pre")
                    for k in range(KH):
                        nc.tensor.matmul(
                            ps[:],
                            xg[:, 512 * k + 128 * g4:512 * k + 128 * (g4 + 1)],
                            lstm_wihT[:, GC * k:GC * (k + 1)],
                            start=(k == 0), stop=(k == KH - 1))
                    stg = prep.tile([128, GC], F32, tag="stg", name="stg")
                    nc.vector.scalar_tensor_tensor(
                        stg[:], ps[:], 0.0, lstm_bias, Alu.add, Alu.add)
                    nc.sync.dma_start(prex_d[g], stg[:])

        # ================= decoder =================
        nc.sync.dma_start(hT_all[:], io["h0T"])
        nc.sync.dma_start(c_chunk[:], io["c0c"])

        for t in range(T):
            lstm_step(
                t, lstm_whhT, prex_d,
                h_bp_dest=h_bp_fin[:],
                arch_T_dest=arch_decT[:, B * t:B * (t + 1)],
                do_ag=(t < T - 1),
            )

        # ================= outputs =================
        with tc.tile_pool(name="tail", bufs=1) as tailp:
            q_sb = tailp.tile([A, T * B], F32, tag="q_sb", name="q_sb")
            for n in range(T * B // 512):
                ps_q = ptile([A, 512], "ps_q")
                nc.tensor.matmul(
                    ps_q[:], wqoT, arch_decT[:, 512 * n:512 * (n + 1)],
                    start=True, stop=True)
                nc.vector.tensor_copy(q_sb[:, 512 * n:512 * (n + 1)], ps_q[:])
            nc.sync.dma_start(io["q_part"], q_sb[:])
            h_outT = tailp.tile([HC, B], F32, tag="h_outT", name="h_outT")
            nc.vector.tensor_copy(h_outT[:], arch_decT[:, B * (T - 1):B * T])
            nc.sync.dma_start(io["h_out"], h_outT[:])
            nc.sync.dma_start(io["c_out"], c_chunk[:])

    nc.compile()
    return nc


def _get_program():
    if "nc" not in _CACHE:
        _CACHE["nc"] = _build_program()
    return _CACHE["nc"]


# ---------------------------------------------------------------- entry point

def kernel(**inputs):
    in_maps, b_qo = _prep_inputs(inputs)
    nc = _get_program()

    if os.environ.get("KSIM"):
        from concourse.bass_interp import MultiCoreSim
        sim = MultiCoreSim(nc, NC, trace=False)
        for c in range(NC):
            for name, arr in in_maps[c].items():
                sim.cores[c].tensor(name)[:] = arr
        sim.simulate(check_with_hw=False)
        _CACHE["sim_time_ns"] = sim.global_time
        print(f"sim est exec time: {sim.global_time} ns")
        results = [
            {n: np.array(sim.cores[c].mem_tensor(n)) for n, _, _ in _OUTPUT_SPECS}
            for c in range(NC)
        ]
    else:
        from concourse.bass_utils import run_bass_kernel_spmd
        res = run_bass_kernel_spmd(nc, in_maps, core_ids=list(range(NC)))
        results = res.results
        _CACHE["last_results"] = res

    q = sum(r["q_part"].astype(np.float64) for r in results)
    q = (q.reshape(A, T, B).transpose(1, 2, 0) + b_qo[None, None, :])
    q = np.ascontiguousarray(q.astype(np.float32))
    hT = np.concatenate([r["h_out"] for r in results], axis=0)
    h = np.ascontiguousarray(hT.T.astype(np.float32))
    c = np.concatenate([r["c_out"] for r in results], axis=1)
    return q, h, np.ascontiguousarray(c.astype(np.float32))


# revision 15
# speedup vs baseline: 1.0038x; 1.0038x over previous
"""Trainium2 Bass kernel for nn_DemonstrationAttentionQModel.

Key algebraic facts exploited (all exact):
  - Only demonstration[0] and demonstration_length[0] are used.
  - scores = enc@Wd + b + (h@Wh) : the h term is a per-batch scalar added
    uniformly across L, and softmax is shift-invariant => attention weights
    are constant over decoder time.
  - Hence decoder inputs x_t = relu(comb([attn, obs_t])) are precomputable;
    only the LSTM h/c recurrences (80 + 40 steps) are sequential.
  - mid/out are linear-linear => fused into one [18,1024] matmul at the end.

Distribution: Megatron-style gate split over 8 cores. Core c owns h-dims
[128c,128c+128) (gate rows reordered [i,f,o,g]); per step it computes its 512
gates (weights as the f32r moving operand, N=512), does the cell elementwise,
PE-transposes its h-chunk and AllGathers the 8 chunks into the full hT.

kernel(**inputs) -> (q [T,B,A], h [B,H], c [B,H]) float32.
"""

import os
import sys

sys.path.insert(0, "/opt/trn_rl_repo")

import numpy as np

import concourse.bass as bass  # noqa
import concourse.bacc as bacc
import concourse.mybir as mybir
import concourse.tile as tile

dt = mybir.dt
Act = mybir.ActivationFunctionType
Alu = mybir.AluOpType
Ax = mybir.AxisListType

NC = 8
T, B, L = 40, 64, 80
DIN = 256
H = 1024
A = 18
HC = H // NC          # 128
GC = 4 * HC           # 512
KH = H // 128         # 8
KD = DIN // 128       # 2
NEG = -1e30

F32 = dt.float32
F32R = dt.float32r

_CACHE = {}


# ---------------------------------------------------------------- host prep

def _gate_rows(c):
    """Gate rows of core c, reordered [i, f, o, g] (PyTorch order is i,f,g,o)."""
    hs = np.arange(HC) + HC * c
    return np.concatenate([hs, H + hs, 3 * H + hs, 2 * H + hs])


def _pack_rhs(w_T):
    """[K, N] -> [128, (K//128)*N], k-tile t at cols [N*t : N*(t+1)]."""
    K, N = w_T.shape
    k = K // 128
    return np.ascontiguousarray(
        w_T.reshape(k, 128, N).transpose(1, 0, 2).reshape(128, k * N)
    )


def _prep_inputs(inputs):
    f = lambda x: np.asarray(x, dtype=np.float32)
    state = f(inputs["state"])                     # [T,B,DIN]
    demo0 = f(inputs["demonstration"][0])          # [B,L,DIN]
    lengths = np.asarray(inputs["demonstration_length"][0], dtype=np.int64)
    h0, c0 = f(inputs["h0"]), f(inputs["c0"])

    enc_Wih, enc_Whh = f(inputs["enc_Wih"]), f(inputs["enc_Whh"])
    enc_b = f(inputs["enc_bih"]) + f(inputs["enc_bhh"])
    attn_W, attn_b = f(inputs["attn_W"]), f(inputs["attn_b"])
    comb_W, comb_b = f(inputs["comb_W"]), f(inputs["comb_b"])
    lstm_Wih, lstm_Whh = f(inputs["lstm_Wih"]), f(inputs["lstm_Whh"])
    lstm_b = f(inputs["lstm_bih"]) + f(inputs["lstm_bhh"])
    mid_W, mid_b = f(inputs["mid_W"]), f(inputs["mid_b"])
    out_W, out_b = f(inputs["out_W"]), f(inputs["out_b"])

    W_qo = out_W @ mid_W                   # [A, H]
    b_qo = out_W @ mid_b + out_b           # [A]

    xe = demo0.transpose(1, 0, 2).reshape(L * B, DIN)      # (l*B+b, DIN)
    xe_T = _pack_rhs(np.ascontiguousarray(xe.T))
    st = state.reshape(T * B, DIN)                         # (t*B+b, DIN)
    st_T = _pack_rhs(np.ascontiguousarray(st.T))
    penalty = np.where(
        np.arange(L)[None, :] < lengths[:, None], attn_b[0], NEG
    ).astype(np.float32)                                    # [B,L]
    h0T = np.ascontiguousarray(h0.T)                        # [H,B]
    Wd = attn_W[0, :H]

    in_maps = []
    for c in range(NC):
        rows = _gate_rows(c)
        hs = slice(HC * c, HC * (c + 1))
        in_maps.append({
            "enc_whhT": _pack_rhs(np.ascontiguousarray(enc_Whh[rows].T)),
            "lstm_whhT": _pack_rhs(np.ascontiguousarray(lstm_Whh[rows].T)),
            "enc_wihT": _pack_rhs(np.ascontiguousarray(enc_Wih[rows].T)),
            "lstm_wihT": _pack_rhs(np.ascontiguousarray(lstm_Wih[rows].T)),
            "combT": _pack_rhs(np.ascontiguousarray(comb_W[hs].T)),
            "wqoT": np.ascontiguousarray(W_qo[:, hs].T),
            "wd": np.ascontiguousarray(Wd[hs][:, None]),
            "enc_bias": np.broadcast_to(enc_b[rows], (128, GC)).copy(),
            "lstm_bias": np.broadcast_to(lstm_b[rows], (128, GC)).copy(),
            "comb_bias": np.ascontiguousarray(comb_b[hs][:, None]),
            "penalty": penalty,
            "xe_T": xe_T,
            "st_T": st_T,
            "h0T": _pack_rhs(h0T),
            "c0c": np.ascontiguousarray(c0[:, hs]),
            "ident": np.eye(128, dtype=np.float32),
        })
    return in_maps, b_qo


# ------------------------------------------------------------- device program

_INPUT_SPECS = [
    ("enc_whhT", [128, KH * GC], F32R),
    ("lstm_whhT", [128, KH * GC], F32R),
    ("enc_wihT", [128, KD * GC], F32R),
    ("lstm_wihT", [128, KH * GC], F32R),
    ("combT", [128, 10 * HC], F32R),
    ("wqoT", [HC, A], F32R),
    ("wd", [HC, 1], F32R),
    ("enc_bias", [128, GC], F32),
    ("lstm_bias", [128, GC], F32),
    ("comb_bias", [HC, 1], F32),
    ("penalty", [B, L], F32),
    ("xe_T", [128, KD * L * B], F32R),
    ("st_T", [128, KD * T * B], F32R),
    ("h0T", [128, KH * B], F32R),
    ("c0c", [B, HC], F32),
    ("ident", [128, 128], F32),
]

_OUTPUT_SPECS = [
    ("q_part", [A, T * B], F32),
    ("h_out", [HC, B], F32),
    ("c_out", [B, HC], F32),
]

RG = [list(range(NC))]


def _build_program():
    nc = bacc.Bacc("TRN2", target_bir_lowering=False, debug=False,
                   num_devices=NC)
    io = {}
    for name, shape, d in _INPUT_SPECS:
        io[name] = nc.dram_tensor(name, shape, d, kind="ExternalInput").ap()
    for name, shape, d in _OUTPUT_SPECS:
        io[name] = nc.dram_tensor(name, shape, d, kind="ExternalOutput").ap()

    # internal DRAM
    prei_d = nc.dram_tensor("prei_d", [L * B // 128, 128, GC], F32)
    prex_d = nc.dram_tensor("prex_d", [T * B // 128, 128, GC], F32)
    scores_d = nc.dram_tensor("scores_d", [L, B], F32)
    scores_r = nc.dram_tensor("scores_r", [L, B], F32, addr_space="Shared")
    arch_d = nc.dram_tensor("arch_d", [B, L, HC], F32)

    import contextlib
    with tile.TileContext(nc) as tc, contextlib.ExitStack() as ctx:
        persist = ctx.enter_context(tc.tile_pool(name="persist", bufs=1))
        psum = ctx.enter_context(tc.tile_pool(name="psum", bufs=4, space="PSUM"))
        cell = ctx.enter_context(tc.tile_pool(name="cell", bufs=8))
        prep = ctx.enter_context(tc.tile_pool(name="prep", bufs=3))
        dram = ctx.enter_context(tc.tile_pool(name="dram", bufs=3, space="DRAM"))

        def ptile(shape, name):
            return psum.tile(shape, F32, tag="ps", name=name)

        def ctile(shape, name, dtype=F32):
            return cell.tile(shape, dtype, tag="cl", name=name)

        # --- persistent smalls: two merged tiles + per-core state ---
        # smalls_r (f32r): wd [HC,1] at col 0; wqoT [HC,A] at cols 1:1+A
        smalls_r = persist.tile([128, 1 + A], F32R, tag="smalls_r",
                                name="smalls_r")
        nc.sync.dma_start(smalls_r[:HC, 0:1], io["wd"])
        nc.sync.dma_start(smalls_r[:HC, 1:1 + A], io["wqoT"])
        wd = smalls_r[:HC, 0:1]
        wqoT = smalls_r[:HC, 1:1 + A]
        # smalls_f (f32): comb_bias 1 | ident 128 | penalty 80 | enc_bias 512
        #                 | lstm_bias 512
        smalls_f = persist.tile([128, 1 + 128 + L + GC + GC], F32,
                                tag="smalls_f", name="smalls_f")
        nc.sync.dma_start(smalls_f[:HC, 0:1], io["comb_bias"])
        nc.sync.dma_start(smalls_f[:, 1:129], io["ident"])
        nc.sync.dma_start(smalls_f[:B, 129:129 + L], io["penalty"])
        nc.sync.dma_start(smalls_f[:, 209:209 + GC], io["enc_bias"])
        nc.sync.dma_start(smalls_f[:, 209 + GC:209 + 2 * GC], io["lstm_bias"])
        comb_bias = smalls_f[:HC, 0:1]
        ident = smalls_f[:, 1:129]
        penalty = smalls_f[:B, 129:129 + L]
        enc_bias = smalls_f[:, 209:209 + GC]
        lstm_bias = smalls_f[:, 209 + GC:209 + 2 * GC]

        hT_all = persist.tile([128, KH * B], F32R, tag="hT_all", name="hT_all")
        c_chunk = persist.tile([B, HC], F32, tag="c_chunk", name="c_chunk")
        attnT = persist.tile([128, KH * B], F32R, tag="attnT", name="attnT")
        arch_decT = persist.tile([HC, T * B], F32R, tag="arch_decT",
                                 name="arch_decT")
        h_bp_fin = persist.tile([B, HC], F32, tag="h_bp_fin", name="h_bp_fin")

        def lstm_step(step, whhT, pre_d, h_bp_dest, arch_T_dest, do_ag,
                      score_dst=None, arch_dram=None):
            g, off = divmod(step, 2)
            pre = prep.tile([B, GC], F32, tag="pre", name="pre")
            nc.sync.dma_start(pre[:], pre_d[g, 64 * off:64 * off + B, :])
            psum_g = ptile([B, GC], "psum_g")
            for k in range(KH):
                nc.tensor.matmul(
                    psum_g[:], hT_all[:, B * k:B * (k + 1)],
                    whhT[:, GC * k:GC * (k + 1)],
                    start=(k == 0), stop=(k == KH - 1),
                )
            gsum = ctile([B, GC], "gsum")
            nc.vector.scalar_tensor_tensor(
                gsum[:], psum_g[:], 0.0, pre[:], Alu.add, Alu.add)
            sifo = ctile([B, 3 * HC], "sifo")
            nc.scalar.activation(sifo[:], gsum[:, :3 * HC], Act.Sigmoid)
            tg = ctile([B, HC], "tg")
            nc.scalar.activation(tg[:], gsum[:, 3 * HC:], Act.Tanh)
            t1 = ctile([B, HC], "t1")
            nc.vector.scalar_tensor_tensor(
                t1[:], sifo[:, HC:2 * HC], 0.0, c_chunk[:], Alu.add, Alu.mult)
            t2 = ctile([B, HC], "t2")
            nc.vector.scalar_tensor_tensor(
                t2[:], sifo[:, :HC], 0.0, tg[:], Alu.add, Alu.mult)
            nc.vector.scalar_tensor_tensor(
                c_chunk[:], t1[:], 0.0, t2[:], Alu.add, Alu.add)
            tc_ = ctile([B, HC], "tc_")
            nc.scalar.activation(tc_[:], c_chunk[:], Act.Tanh)
            nc.vector.scalar_tensor_tensor(
                h_bp_dest, sifo[:, 2 * HC:], 0.0, tc_[:], Alu.add, Alu.mult)
            if arch_dram is not None:
                nc.sync.dma_start(arch_dram, h_bp_dest)
            psum_hT = ptile([HC, B], "psum_hT")
            nc.tensor.transpose(psum_hT[:], h_bp_dest, ident[:B, :B])
            nc.vector.tensor_copy(arch_T_dest, psum_hT[:])
            if score_dst is not None:
                psum_sc = ptile([1, B], "psum_sc")
                nc.tensor.matmul(psum_sc[:], wd, arch_T_dest,
                                 start=True, stop=True)
                sc_sb = ctile([1, B], "sc_sb")
                nc.vector.tensor_copy(sc_sb[:], psum_sc[:])
                nc.sync.dma_start(score_dst, sc_sb[:])
            if do_ag:
                cc_in = dram.tile([HC, B], F32R, tag="cc_in", name="cc_in")
                cc_out = dram.tile([H, B], F32R, tag="cc_out", name="cc_out",
                                   addr_space="Shared")
                nc.sync.dma_start(cc_in[:], arch_T_dest)
                nc.gpsimd.collective_compute(
                    "AllGather", Alu.bypass, ins=[cc_in.opt()],
                    outs=[cc_out.opt()], replica_groups=RG)
                nc.sync.dma_start(
                    hT_all[:].rearrange("p (k n) -> p k n", n=B),
                    cc_out[:].rearrange("(k p) n -> p k n", p=128))

        # ================= encoder =================
        with tc.tile_pool(name="enc_phase", bufs=1) as encp:
            enc_whhT = encp.tile([128, KH * GC], F32R, tag="enc_whhT",
                                 name="enc_whhT")
            nc.sync.dma_start(enc_whhT[:], io["enc_whhT"])

            # --- PreI -> DRAM ---
            with tc.tile_pool(name="xe_pool", bufs=1) as xep:
                enc_wihT = xep.tile([128, KD * GC], F32R, tag="enc_wihT",
                                    name="enc_wihT")
                nc.sync.dma_start(enc_wihT[:], io["enc_wihT"])
                xe_sb = xep.tile([128, KD * L * B], F32R, tag="xe_sb",
                                 name="xe_sb")
                nc.sync.dma_start(xe_sb[:], io["xe_T"])
                for g in range(L * B // 128):            # 40 groups
                    ps = ptile([128, GC], "ps_pre")
                    for k in range(KD):
                        nc.tensor.matmul(
                            ps[:],
                            xe_sb[:, L * B * k + 128 * g:L * B * k + 128 * (g + 1)],
                            enc_wihT[:, GC * k:GC * (k + 1)],
                            start=(k == 0), stop=(k == KD - 1),
                        )
                    stg = prep.tile([128, GC], F32, tag="stg", name="stg")
                    nc.vector.scalar_tensor_tensor(
                        stg[:], ps[:], 0.0, enc_bias, Alu.add, Alu.add)
                    nc.sync.dma_start(prei_d[g], stg[:])

            # --- encoder recurrence ---
            nc.vector.memset(hT_all[:].bitcast(F32), 0.0)
            nc.vector.memset(c_chunk[:], 0.0)
            for l in range(L):
                stageT = ctile([HC, B], "stageT", F32R)
                h_bp = ctile([B, HC], "h_bp")
                lstm_step(
                    l, enc_whhT, prei_d,
                    h_bp_dest=h_bp[:],
                    arch_T_dest=stageT[:],
                    do_ag=(l < L - 1),
                    score_dst=scores_d[l],
                    arch_dram=arch_d[:, l, :],
                )

            # --- scores AllReduce + softmax ---
            with tc.tile_pool(name="smx", bufs=4) as smx:
                def stile(shape, name):
                    return smx.tile(shape, F32, tag="sm", name=name)

                nc.gpsimd.collective_compute(
                    "AllReduce", Alu.add, ins=[scores_d.ap().opt()],
                    outs=[scores_r.ap().opt()], replica_groups=RG)
                sc_lb = stile([L, B], "sc_lb")
                nc.sync.dma_start(sc_lb[:], scores_r.ap())
                ps_sc = ptile([B, L], "ps_scT")
                nc.tensor.transpose(ps_sc[:], sc_lb[:], ident[:L, :L])
                scores_bp = stile([B, L], "scores_bp")
                nc.vector.scalar_tensor_tensor(
                    scores_bp[:], ps_sc[:], 0.0, penalty, Alu.add, Alu.add)
                mx = stile([B, 1], "mx")
                nc.vector.tensor_reduce(mx[:], scores_bp[:], Ax.X, Alu.max)
                nmx = stile([B, 1], "nmx")
                nc.vector.tensor_scalar_mul(nmx[:], mx[:], -1.0)
                wexp = stile([B, L], "wexp")
                sumexp = stile([B, 1], "sumexp")
                nc.scalar.activation(wexp[:], scores_bp[:], Act.Exp,
                                     bias=nmx[:], scale=1.0,
                                     accum_out=sumexp[:])
                rsum = stile([B, 1], "rsum")
                nc.vector.reciprocal(rsum[:], sumexp[:])
                wat = stile([B, L], "wat")
                nc.vector.tensor_scalar(wat[:], wexp[:], rsum[:], None,
                                        Alu.mult)

                # --- attention in L-chunks from the DRAM archive ---
                LC = 20
                attn_acc = stile([B, HC], "attn_acc")
                for q in range(L // LC):
                    aq = smx.tile([B, LC * HC], F32, tag="aq", name="aq",
                                  bufs=2)
                    nc.sync.dma_start(
                        aq[:].rearrange("b (l h) -> b l h", l=LC),
                        arch_d[:, LC * q:LC * (q + 1), :])
                    pq = smx.tile([B, LC * HC], F32, tag="pq", name="pq",
                                  bufs=2)
                    nc.vector.scalar_tensor_tensor(
                        pq[:].rearrange("b (h l) -> b l h", l=LC),
                        aq[:].rearrange("b (l h) -> b l h", l=LC),
                        0.0,
                        wat[:, LC * q:LC * (q + 1)].rearrange(
                            "b (l o) -> b l o", o=1).broadcast_to([B, LC, HC]),
                        Alu.add, Alu.mult)
                    part = stile([B, HC], "attn_part")
                    nc.vector.tensor_reduce(
                        part[:], pq[:].rearrange("b (h l) -> b h l", l=LC),
                        Ax.X, Alu.add)
                    if q == 0:
                        nc.vector.tensor_copy(attn_acc[:], part[:])
                    else:
                        nc.vector.scalar_tensor_tensor(
                            attn_acc[:], attn_acc[:], 0.0, part[:],
                            Alu.add, Alu.add)
                ps_at = ptile([HC, B], "ps_at")
                nc.tensor.transpose(ps_at[:], attn_acc[:], ident[:B, :B])
                attn_stage = smx.tile([HC, B], F32R, tag="sm",
                                      name="attn_stage")
                nc.vector.tensor_copy(attn_stage[:], ps_at[:])
                at_in = dram.tile([HC, B], F32R, tag="at_in", name="at_in")
                at_out = dram.tile([H, B], F32R, tag="at_out", name="at_out",
                                   addr_space="Shared")
                nc.sync.dma_start(at_in[:], attn_stage[:])
                nc.gpsimd.collective_compute(
                    "AllGather", Alu.bypass, ins=[at_in.opt()],
                    outs=[at_out.opt()], replica_groups=RG)
                nc.sync.dma_start(
                    attnT[:].rearrange("p (k n) -> p k n", n=B),
                    at_out[:].rearrange("(k p) n -> p k n", p=128))

        # ================= decoder-side weights =================
        lstm_whhT = persist.tile([128, KH * GC], F32R, tag="lstm_whhT",
                                 name="lstm_whhT")
        nc.sync.dma_start(lstm_whhT[:], io["lstm_whhT"])

        # ================= x / PreX precompute =================
        x_out = dram.tile([H, T * B], F32R, tag="x_out", name="x_out",
                      addr_space="Shared")
        with tc.tile_pool(name="x_pool", bufs=1) as xp:
            combT = xp.tile([128, 10 * HC], F32R, tag="combT", name="combT")
            nc.sync.dma_start(combT[:], io["combT"])
            st_sb = xp.tile([128, KD * T * B], F32R, tag="st_sb", name="st_sb")
            nc.sync.dma_start(st_sb[:], io["st_T"])
            ps_pa = ptile([HC, B], "ps_pa")
            for k in range(KH):
                nc.tensor.matmul(
                    ps_pa[:], combT[:, HC * k:HC * (k + 1)],
                    attnT[:, B * k:B * (k + 1)],
                    start=(k == 0), stop=(k == KH - 1))
            part_a = xp.tile([HC, B], F32, tag="part_a", name="part_a")
            nc.vector.tensor_copy(part_a[:], ps_pa[:])
            xT_sb = xp.tile([HC, T * B], F32R, tag="xT_sb", name="xT_sb")
            for n in range(T * B // 512):                # 5 chunks
                ps_x = ptile([HC, 512], "ps_x")
                for k in range(KD):
                    nc.tensor.matmul(
                        ps_x[:],
                        combT[:, HC * (KH + k):HC * (KH + k + 1)],
                        st_sb[:, T * B * k + 512 * n:T * B * k + 512 * (n + 1)],
                        start=(k == 0), stop=(k == KD - 1))
                xsum = xp.tile([HC, 512], F32, tag="xsum", name="xsum", bufs=2)
                nc.vector.scalar_tensor_tensor(
                    xsum[:].rearrange("p (t b) -> p t b", b=B),
                    ps_x[:].rearrange("p (t b) -> p t b", b=B),
                    0.0,
                    part_a[:].rearrange("p (o b) -> p o b", o=1).broadcast_to(
                        [HC, 512 // B, B]),
                    Alu.add, Alu.add)
                nc.scalar.activation(
                    xT_sb[:, 512 * n:512 * (n + 1)], xsum[:], Act.Relu,
                    bias=comb_bias, scale=1.0)
            x_in = dram.tile([HC, T * B], F32R, tag="x_in", name="x_in")
            nc.sync.dma_start(x_in[:], xT_sb[:])
            nc.gpsimd.collective_compute(
                "AllGather", Alu.bypass, ins=[x_in.opt()], outs=[x_out.opt()],
                replica_groups=RG)

        with tc.tile_pool(name="px_pool", bufs=1) as pxp:
            lstm_wihT = pxp.tile([128, KH * GC], F32R, tag="lstm_wihT",
                                 name="lstm_wihT")
            nc.sync.dma_start(lstm_wihT[:], io["lstm_wihT"])
            # PreX in 5 supergroups of 4 M-groups (512 (t,b) rows each)
            for sg in range(T * B // 512):
                xg = pxp.tile([128, KH * 512], F32R, tag="xg", name="xg",
                              bufs=2)
                nc.sync.dma_start(
                    xg[:].rearrange("p (k n) -> p k n", n=512),
                    x_out[:, 512 * sg:512 * (sg + 1)].rearrange(
                        "(k p) n -> p k n", p=128))
                for g4 in range(4):
                    g = 4 * sg + g4
                    ps = ptile([128, GC], "ps_pre")
                    for k in range(KH):
                        nc.tensor.matmul(
                            ps[:],
                            xg[:, 512 * k + 128 * g4:512 * k + 128 * (g4 + 1)],
                            lstm_wihT[:, GC * k:GC * (k + 1)],
                            start=(k == 0), stop=(k == KH - 1))
                    stg = prep.tile([128, GC], F32, tag="stg", name="stg")
                    nc.vector.scalar_tensor_tensor(
                        stg[:], ps[:], 0.0, lstm_bias, Alu.add, Alu.add)
                    nc.sync.dma_start(prex_d[g], stg[:])

        # ================= decoder =================
        nc.sync.dma_start(hT_all[:], io["h0T"])
        nc.sync.dma_start(c_chunk[:], io["c0c"])

        for t in range(T):
            lstm_step(
                t, lstm_whhT, prex_d,
                h_bp_dest=h_bp_fin[:],
                arch_T_dest=arch_decT[:, B * t:B * (t + 1)],
                do_ag=(t < T - 1),
            )

        # ================= outputs =================
        with tc.tile_pool(name="tail", bufs=1) as tailp:
            q_sb = tailp.tile([A, T * B], F32, tag="q_sb", name="q_sb")
            for n in range(T * B // 512):
                ps_q = ptile([A, 512], "ps_q")
                nc.tensor.matmul(
                    ps_q[:], wqoT, arch_decT[:, 512 * n:512 * (n + 1)],
                    start=True, stop=True)
                nc.vector.tensor_copy(q_sb[:, 512 * n:512 * (n + 1)], ps_q[:])
            nc.sync.dma_start(io["q_part"], q_sb[:])
            h_outT = tailp.tile([HC, B], F32, tag="h_outT", name="h_outT")
            nc.vector.tensor_copy(h_outT[:], arch_decT[:, B * (T - 1):B * T])
            nc.sync.dma_start(io["h_out"], h_outT[:])
            nc.sync.dma_start(io["c_out"], c_chunk[:])

    nc.compile()
    return nc


def _get_program():
    if "nc" not in _CACHE:
        _CACHE["nc"] = _build_program()
    return _CACHE["nc"]


# ---------------------------------------------------------------- entry point

def kernel(**inputs):
    in_maps, b_qo = _prep_inputs(inputs)
    nc = _get_program()

    if os.environ.get("KSIM"):
        from concourse.bass_interp import MultiCoreSim
        sim = MultiCoreSim(nc, NC, trace=False)
        for c in range(NC):
            for name, arr in in_maps[c].items():
                sim.cores[c].tensor(name)[:] = arr
        sim.simulate(check_with_hw=False)
        _CACHE["sim_time_ns"] = sim.global_time
        print(f"sim est exec time: {sim.global_time} ns")
        results = [
            {n: np.array(sim.cores[c].mem_tensor(n)) for n, _, _ in _OUTPUT_SPECS}
            for c in range(NC)
        ]
    else:
        from concourse.bass_utils import run_bass_kernel_spmd
        res = run_bass_kernel_spmd(nc, in_maps, core_ids=list(range(NC)))
        results = res.results
        _CACHE["last_results"] = res

    q = sum(r["q_part"].astype(np.float64) for r in results)
    q = (q.reshape(A, T, B).transpose(1, 2, 0) + b_qo[None, None, :])
    q = np.ascontiguousarray(q.astype(np.float32))
    hT = np.concatenate([r["h_out"] for r in results], axis=0)
    h = np.ascontiguousarray(hT.T.astype(np.float32))
    c = np.concatenate([r["c_out"] for r in results], axis=1)
    return q, h, np.ascontiguousarray(c.astype(np.float32))
